# revision 43
# baseline (speedup 1.0000x reference)
"""Trainium2 Bass kernel for the AttentionLayer problem.

Math (per batch):
    Q = inp_q @ Wq + bq            [S, d]
    K = inp_k @ Wk + bk            [S, d]
    V = inp_v @ Wv + bv            [S, d]
    sc = Q @ K^T / sqrt(d)         [Sq, Sk]
    S_ = softmax(sc, axis=0)       (over the QUERY axis)
    H = S_ @ V                     [Sq, d]

Schedule (per core, 2 batches, fully software-pipelined):
  The exp chain on ACT (~3.3us per 128-key chunk, 106us/core) and the
  matmul stream on PE (~117us/core at the power-throttled 2.0GHz clock)
  are the two near-critical engines; every phase of batch b is emitted
  interleaved with phases of the other batch so both engines stay busy:

    PE:  [Qproj b0 | kslabs b0 + sc b0(0..8) + Vnat b0 | kslabs b1 +
          sc b0(9..12) | Qproj b1 + sc b0(13..15) | sc b1 + Vnat b1 +
          H b0 | H b1 ]
    ACT: [exp b0 chunks 0..15 | exp b1 chunks 0..15 | last out copy]
    DMA: q0, k0, v0, k1, q1, v1 (the order activations are consumed)

  PSUM (8 banks total):
    P1 (2 x [128,1024] f32 = 4 banks): Qproj-b0 accum halves, then the
       rotating double-buffered scores tiles for both batches.
    P2 (2 x 4KB slots = 4 banks): kps slabs b0 -> V-natural accum b0 ->
       kps slabs b1 -> Qproj-b1 accum halves -> V-natural accum b1 ->
       H accum tiles (one [128,1024] per q-half, both live at once).

  V is projected directly in natural [key, d] layout (lhsT = x-slice,
  ap=128 matmuls: LDWEIGHTS hides under FWL), drained UNNORMALIZED to
  SBUF early (frees PSUM for the next phase), then normalized per key
  chunk k by 1/Z[k] on DVE once chunk k's exp-sum is known.
  H^T[d,q] accumulates k-outer with both q-half tiles live so only the
  last key chunk's matmuls trail the final exp.
Compute dtype bf16 (f32 PSUM accumulate), stats in f32.
"""

import math
import sys

sys.path.insert(0, "/opt/trn_rl_repo")

import ml_dtypes
import numpy as np

BF16_NP = ml_dtypes.bfloat16

import concourse.bass as bass  # noqa: E402
import concourse.tile as tile  # noqa: E402
from concourse import bacc, mybir  # noqa: E402

P = 128          # partitions / head dim d
S = 2048         # sequence length
D = 1024         # model dim
DC = D // P      # D chunks (8)
KC = S // P      # key chunks (16)
B_LOC = 2        # batches per core
N_CORES = 8
SCALE = 1.0 / math.sqrt(P)

F32 = mybir.dt.float32
BF16 = mybir.dt.bfloat16
EXP = mybir.ActivationFunctionType.Exp
COPY = mybir.ActivationFunctionType.Copy

_BUILT = None  # cached (nc,) so repeated kernel() calls reuse the NEFF


def build():
    nc = bacc.Bacc("TRN2", target_bir_lowering=False, debug=False,
                   num_devices=N_CORES)

    dr_in = {}
    dr_in["v"] = nc.dram_tensor("vT", [B_LOC, D, S], BF16,
                                kind="ExternalInput")
    # q host-packed per 512-column s-block: [b][sb][p][c*512+j] =
    # q[b][sb*512+j][c*128+p].  s-major blocks let the Q projection
    # finish its first 1024 output columns after only 1MB of q DMA,
    # so the exp spine starts ~10us earlier than with c-major slabs.
    dr_in["q"] = nc.dram_tensor("qT", [B_LOC, 4, P, DC * 512],
                                BF16, kind="ExternalInput")
    # k host-packed per 256-column slab: [b][sl][p][c*256+j] =
    # k[b][sl*256+j][c*128+p] so each partition reads one contiguous run
    dr_in["k"] = nc.dram_tensor("kT", [B_LOC, KC // 2, P, DC * 256],
                                BF16, kind="ExternalInput")
    dr_w = {t: nc.dram_tensor(f"w{t}", [P, DC * P], BF16,
                              kind="ExternalInput")
            for t in ("q", "k", "v")}
    # biases packed [P, 3] host-side: one DMA instead of three
    # 128-descriptor transfers on the sync queue's critical prefix
    dr_b3 = nc.dram_tensor("b3", [P, 3], F32, kind="ExternalInput")
    dr_bv = nc.dram_tensor("bvr", [P], F32, kind="ExternalInput")
    dr_out = nc.dram_tensor("out", [B_LOC, P, S], BF16, kind="ExternalOutput")

    with tile.TileContext(nc) as tc:
        with (
            tc.tile_pool(name="const", bufs=1) as const,
            tc.tile_pool(name="stream", bufs=4) as stream,
            tc.tile_pool(name="kctp", bufs=16) as kctp,
            tc.tile_pool(name="qtp", bufs=2) as qtp,
            tc.tile_pool(name="ptp", bufs=32) as ptp,
            tc.tile_pool(name="vrawp", bufs=2) as vrawp,
            tc.tile_pool(name="vsp", bufs=32) as vsp,
            tc.tile_pool(name="zzp", bufs=8) as zzp,
            tc.tile_pool(name="recp", bufs=32) as recp,
            tc.tile_pool(name="osb", bufs=2) as osb,
            tc.tile_pool(name="p1", bufs=2, space="PSUM") as p1,
            tc.tile_pool(name="p2", bufs=2, space="PSUM") as p2,
        ):
            w_sb = {}
            for t in ("q", "k", "v"):
                w_sb[t] = const.tile([P, DC, P], BF16, tag=f"w{t}",
                                     name=f"w{t}")
            b3_sb = const.tile([P, 3], F32, tag="b3", name="b3")
            b_sb = {t: b3_sb[:, i:i + 1]
                    for i, t in enumerate(("q", "k", "v"))}

            def load_biases():
                nc.sync.dma_start(b3_sb[:], dr_b3.ap())

            _w_loaded = set()

            def ensure_w(t, eng=None):
                if t in _w_loaded:
                    return
                _w_loaded.add(t)
                (eng or nc.gpsimd).dma_start(
                    w_sb[t][:],
                    dr_w[t].ap().rearrange("p (c e) -> p c e", e=P))

            # V bias as a rank-1 matmul (ones[1,128].T @ bias_row[1,128])
            _vbias_box = []

            def ensure_vbias():
                if not _vbias_box:
                    ones_row = const.tile([1, P], BF16, tag="ones",
                                          name="ones_row")
                    nc.vector.memset(ones_row[:], 1.0)
                    bv_row = const.tile([1, P], BF16, tag="bvr",
                                        name="bv_row")
                    nc.gpsimd.dma_start(
                        bv_row[:],
                        dr_bv.ap().rearrange("(o e) -> o e", o=1))
                    _vbias_box.append((ones_row, bv_row))
                return _vbias_box[0]

            def load_dbl(t, b, cc):
                """One [128, 2, 2048] double D-chunk of v (1MB)."""
                x = stream.tile([P, 2, S], BF16, tag="stream", name="x")
                nc.gpsimd.dma_start(
                    x[:],
                    dr_in[t].ap()[b, cc * 2 * P:(cc + 1) * 2 * P, :]
                    .rearrange("(two p) s -> p two s", two=2))
                return x

            def load_qsb(b, sb, eng=None):
                """One [128, 8, 512] s-block of q (1MB)."""
                x = stream.tile([P, DC, 512], BF16, tag="stream",
                                name="xq")
                (eng or nc.gpsimd).dma_start(
                    x[:],
                    dr_in["q"].ap()[b, sb]
                    .rearrange("p (c s) -> p c s", s=512))
                return x

            # ---------------- Q projection (s-block streamed) ----------
            def emit_qproj_half(b, h, pool, tag, qt, drain_eng,
                                engs=(None, None)):
                """One [128,1024] output half = two 1MB s-blocks, each
                8 accumulating ap512 MMs, then a fused bias-add drain."""
                ensure_w("q")
                acc = pool.tile([P, 1024], F32, tag=tag, name="qacc")
                xs = [load_qsb(b, h * 2 + p_, engs[p_])
                      for p_ in range(2)]
                for p_ in range(2):
                    for c in range(DC):
                        nc.tensor.matmul(
                            acc[:, p_ * 512:(p_ + 1) * 512],
                            lhsT=w_sb["q"][:, c, :],
                            rhs=xs[p_][:, c, :],
                            start=(c == 0), stop=(c == DC - 1))
                sl = qt[:, h * 1024:(h + 1) * 1024]
                if drain_eng == "act":
                    nc.scalar.activation(
                        sl, acc[:],
                        func=mybir.ActivationFunctionType.Identity,
                        bias=b_sb["q"][:])
                else:
                    nc.vector.tensor_scalar_add(sl, acc[:], b_sb["q"][:])

            # ---------------- K slab: [d, 256] = 2 key chunks ----------
            def emit_kslab(b, sl):
                ensure_w("k")
                xk = stream.tile([P, DC, 256], BF16, tag="stream",
                                 name="xk")
                nc.gpsimd.dma_start(
                    xk[:],
                    dr_in["k"].ap()[b, sl]
                    .rearrange("p (c s) -> p c s", s=256))
                kps = p2.tile([P, 256], F32, tag="p2", name="kps")
                for c in range(DC):
                    nc.tensor.matmul(
                        kps[:], lhsT=w_sb["k"][:, c, :], rhs=xk[:, c, :],
                        start=(c == 0), stop=(c == DC - 1))
                kct = kctp.tile([P, 256], BF16, tag="kct", name="kct")
                nc.vector.tensor_scalar_add(kct[:], kps[:], b_sb["k"][:])
                return kct

            # ---------------- scores chunk + exp ----------------------
            def emit_sc(b, st, j, halves=(0, 1)):
                """One key chunk j of scores^T + exp + Z accumulate."""
                kct = st.kcts[j // 2]
                lhsT = kct[:, (j % 2) * P:(j % 2 + 1) * P]
                if 0 in halves:
                    pt = ptp.tile([P, S], BF16, tag="pt", name="pt")
                    zz = zzp.tile([P, 2], F32, tag="zz", name="zz")
                    st.pts.append(pt)
                    st.zzs.append(zz)
                else:
                    pt, zz = st.pts[j], st.zzs[j]
                for h in halves:
                    sc = p1.tile([P, 1024], F32, tag="p1", name="sc")
                    for s2 in range(2):
                        nc.tensor.matmul(
                            sc[:, s2 * 512:(s2 + 1) * 512],
                            lhsT=lhsT,
                            rhs=st.qt[:, h * 1024 + s2 * 512:
                                      h * 1024 + (s2 + 1) * 512],
                            start=True, stop=True)
                    nc.scalar.activation(
                        pt[:, h * 1024:(h + 1) * 1024], sc[:],
                        func=EXP, scale=SCALE, accum_out=zz[:, h:h + 1])

            def emit_rec(st):
                """Emit 1/Z for the next pending chunk (DVE)."""
                zz = st.zzs[len(st.recs)]
                rec = recp.tile([P, 1], F32, tag="rec", name="rec")
                nc.vector.tensor_reduce(
                    rec[:], zz[:], axis=mybir.AxisListType.X,
                    op=mybir.AluOpType.add)
                nc.vector.reciprocal(rec[:], rec[:])
                st.recs.append(rec)

            # ---------------- V natural projection ---------------------
            def emit_v_dbl_mms(b, st, cc, x):
                """V-natural MMs for one double D-chunk: 32 ap128 MMs."""
                ensure_w("v")
                for two in range(2):
                    c = cc * 2 + two
                    for g in range(KC):
                        nc.tensor.matmul(
                            st.v_ps[g // 8][:, g % 8, :],
                            lhsT=x[:, two, g * P:(g + 1) * P],
                            rhs=w_sb["v"][:, c, :],
                            start=(c == 0 and (g % 8) % 4 == 0),
                            stop=False)

            def emit_v_bias(st):
                ones_row, bv_row = ensure_vbias()
                for g in range(KC):
                    nc.tensor.matmul(
                        st.v_ps[g // 8][:, g % 8, :],
                        lhsT=ones_row[:], rhs=bv_row[:],
                        start=False, stop=True)

            def emit_v_drain(st):
                """Unnormalized PSUM->SBUF drain (frees P2 early)."""
                vraw = vrawp.tile([P, KC, P], BF16, tag="vraw",
                                  name="vraw")
                for half in range(2):
                    nc.vector.tensor_copy(
                        vraw[:, half * 8:(half + 1) * 8, :],
                        st.v_ps[half][:])
                st.vraw = vraw

            def emit_norm(st, k):
                """vs[k] = vraw[k] * (1/Z[k]) on DVE (4x mode)."""
                while len(st.recs) <= k:
                    emit_rec(st)
                vs = vsp.tile([P, P], BF16, tag="vs", name="vs")
                nc.vector.tensor_scalar_mul(vs[:], st.vraw[:, k, :],
                                            st.recs[k][:])
                st.vss.append(vs)

            # ---------------- H accumulation ---------------------------
            def emit_h_alloc(st):
                st.hts = [p2.tile([P, 1024], F32, tag="p2", name="ht")
                          for _ in range(2)]

            def emit_h_k1(ht, st, qh, k):
                for s2 in range(2):
                    nc.tensor.matmul(
                        ht[:, s2 * 512:(s2 + 1) * 512],
                        lhsT=st.vss[k][:],
                        rhs=st.pts[k][:, qh * 1024 + s2 * 512:
                                      qh * 1024 + (s2 + 1) * 512],
                        start=(k == 0), stop=(k == KC - 1))

            def emit_h_k(st, k):
                for qh in range(2):
                    emit_h_k1(st.hts[qh], st, qh, k)

            def emit_ht_drain(b, ht, qh, engine, out_eng=None):
                out_sb = osb.tile([P, 1024], BF16, tag="osb",
                                  name="out_sb")
                if engine == "act":
                    nc.scalar.activation(out_sb[:], ht[:], func=COPY)
                else:
                    nc.vector.tensor_copy(out_sb[:], ht[:])
                (out_eng or nc.sync).dma_start(
                    dr_out.ap()[b][:, qh * 1024:(qh + 1) * 1024],
                    out_sb[:])

            class St:   # per-batch bookkeeping
                def __init__(self):
                    self.qt = None
                    self.kcts = []
                    self.pts = []
                    self.zzs = []
                    self.recs = []
                    self.v_ps = None
                    self.vraw = None
                    self.vss = []
                    self.hts = None

            st0, st1 = St(), St()

            def sc_emit(st, b):
                """Emit the next pending scores chunk of batch b (4 MMs
                on PE + 2 exps on ACT), plus the lag-2 1/Z on DVE."""
                j = len(st.pts)
                emit_sc(b, st, j)
                if j >= 2:
                    emit_rec(st)

            # ================= EMISSION SEQUENCE =======================
            # Phase A: b0 Q projection from s-major blocks.  The LAST
            # q block rides the (slower but parallel) sync queue so q0
            # completes ~3us before the serial gpsimd stream could
            # deliver it; both qt halves drain on separate engines (ACT
            # idle pre-spine) so the exp spine starts right after.
            ensure_w("q", nc.sync)
            load_biases()   # MUST precede its readers (the qt drains);
            ensure_w("k", nc.sync)  # w_k not consumed until ~24us
            st0.qt = qtp.tile([P, S], BF16, tag="qt", name="qt0")
            emit_qproj_half(0, 0, p1, "p1", st0.qt, "act")
            emit_qproj_half(0, 1, p1, "p1", st0.qt, "dve")
            st0.kcts.append(emit_kslab(0, 0))
            sc_emit(st0, 0)                # chunk 0

            # Phase B: b0 K slabs (DMA-paced) + sc chunks 1..7 (ACT-
            # paced) + V0 natural MMs streaming behind the v0 DMAs.
            # sc chunk j+1's PSUM slot frees when exp j (same half)
            # retires, so sc emissions are spaced to match; all kps
            # tiles are allocated BEFORE the v_ps tiles so the shared
            # P2 slot rotation matches temporal use.
            xv00 = None
            for j in range(1, 8):
                st0.kcts.append(emit_kslab(0, j))
                if j <= 4:
                    sc_emit(st0, 0)        # chunks 1..4
                elif j == 5:
                    # prefetch v0's first double ahead of k slabs 6,7
                    # (their kct is not consumed until chunks 12..15)
                    xv00 = load_dbl("v", 0, 0)
            st0.v_ps = [p2.tile([P, 8, P], F32, tag="p2", name="v_ps")
                        for _ in range(2)]
            for cc in range(4):            # v0 doubles arrive 30..43us
                x = xv00 if cc == 0 else load_dbl("v", 0, cc)
                emit_v_dbl_mms(0, st0, cc, x)
                if cc < 3:
                    sc_emit(st0, 0)        # chunks 5..7
            emit_v_bias(st0)
            emit_v_drain(st0)

            # Phase C: b1 Q projection [47..58us] + sc b0 chunks 8..11,
            # finely interleaved between the four 1MB q1 s-blocks.
            # q1 is loaded BEFORE k1 so PE has dense work here (kct1 is
            # not needed until ~64us); qacc1 tiles WAR the v_ps0 drains.
            st1.qt = qtp.tile([P, S], BF16, tag="qt", name="qt1")
            sc_emit(st0, 0)                # chunk 8
            qacc1 = []
            for h in range(2):
                acc = p2.tile([P, 1024], F32, tag="p2", name="qacc1")
                qacc1.append(acc)
                for p_ in range(2):
                    x = load_qsb(1, h * 2 + p_)
                    for c in range(DC):
                        nc.tensor.matmul(
                            acc[:, p_ * 512:(p_ + 1) * 512],
                            lhsT=w_sb["q"][:, c, :], rhs=x[:, c, :],
                            start=(c == 0), stop=(c == DC - 1))
                    if len(st0.pts) < 12:
                        sc_emit(st0, 0)    # chunks 9,10,11
                nc.vector.tensor_scalar_add(
                    st1.qt[:, h * 1024:(h + 1) * 1024], acc[:],
                    b_sb["q"][:])
            # normalize b0 V rows 0..9 (recs ready well before this
            # point in the DVE stream)
            for k in range(10):
                emit_norm(st0, k)

            # Phase D: b1 K slabs [50..60us] + sc b0 chunks 12,13
            xv10 = None
            for j in range(8):
                st1.kcts.append(emit_kslab(1, j))
                if j in (1, 4):
                    sc_emit(st0, 0)        # chunks 12,13
                elif j == 5:
                    xv10 = load_dbl("v", 1, 0)   # prefetch (as v0's)

            # Phase E: V1 streaming + sc b0 tail + sc b1 head.
            # Order keeps the exp spine seamless across the batch
            # boundary: b1 chunk 0 must be computed right after b0
            # chunk 15's PSUM slot frees.
            x = xv10
            st1.v_ps = [p2.tile([P, 8, P], F32, tag="p2", name="v_ps1")
                        for _ in range(2)]
            emit_v_dbl_mms(1, st1, 0, x)
            sc_emit(st0, 0)                # chunk 14
            x = load_dbl("v", 1, 1)
            emit_v_dbl_mms(1, st1, 1, x)
            sc_emit(st0, 0)                # chunk 15
            sc_emit(st1, 1)                # b1 chunk 0
            x = load_dbl("v", 1, 2)
            emit_v_dbl_mms(1, st1, 2, x)
            sc_emit(st1, 1)                # b1 chunk 1
            x = load_dbl("v", 1, 3)
            emit_v_dbl_mms(1, st1, 3, x)
            emit_v_bias(st1)
            sc_emit(st1, 1)                # b1 chunk 2
            emit_v_drain(st1)
            for k in range(10, KC):        # finish b0 normalizes
                emit_norm(st0, k)

            # Phase F: H0 accumulation split into two q-half passes so
            # the first half's PSUM tile drains (and its output ships)
            # while the second half accumulates — H1's first tile can
            # then allocate mid-F instead of after both drains.
            # b1 sc chunks 3..9 woven at the spine's pace (sc first:
            # ACT is the spine, PE may briefly wait on the scores slot
            # WAR but never starves ACT).
            ht00 = p2.tile([P, 1024], F32, tag="p2", name="ht00")
            for k in range(KC):
                if k in (0, 5, 10):
                    sc_emit(st1, 1)        # b1 chunks 3,4,5
                emit_h_k1(ht00, st0, 0, k)
            emit_ht_drain(0, ht00, 0, "dve")
            ht01 = p2.tile([P, 1024], F32, tag="p2", name="ht01")
            for k in range(KC):
                if k in (0, 4, 8, 12):
                    sc_emit(st1, 1)        # b1 chunks 6..9
                emit_h_k1(ht01, st0, 1, k)
            # b1 norms 0,1 BEFORE the b0 output drain in the DVE
            # stream so H1's start is not delayed behind the copy
            emit_norm(st1, 0)
            emit_norm(st1, 1)
            emit_ht_drain(0, ht01, 1, "dve")

            # Phase G: H1 woven with b1 sc chunks 10..15 (tail).
            # The two final outputs ride different DMA queues so their
            # fixed completion costs overlap.
            emit_h_alloc(st1)
            for k in range(KC):
                if k % 2 == 0 and k < 12:
                    sc_emit(st1, 1)        # b1 chunks 10..15
                if k >= 2:
                    emit_norm(st1, k)
                emit_h_k(st1, k)
            emit_ht_drain(1, st1.hts[0], 0, "dve", nc.gpsimd)
            emit_ht_drain(1, st1.hts[1], 1, "act")

    nc.compile()
    return nc


def _get_nc():
    global _BUILT
    if _BUILT is None:
        _BUILT = build()
    return _BUILT


def pack_w(wk):
    """[D, P] f32 -> [P, DC*P] bf16 in the on-chip [p, c, e] layout."""
    wk = np.asarray(wk, dtype=np.float32)
    return np.ascontiguousarray(
        wk.reshape(DC, P, P).transpose(1, 0, 2).reshape(P, DC * P)
    ).astype(BF16_NP)


def kernel(inp_q, inp_k, inp_v, Wq_kernel, Wq_bias, Wk_kernel, Wk_bias,
           Wv_kernel, Wv_bias):
    from concourse.bass_utils import run_bass_kernel_spmd

    nc = _get_nc()

    inp = {"q": np.asarray(inp_q, dtype=np.float32),
           "k": np.asarray(inp_k, dtype=np.float32),
           "v": np.asarray(inp_v, dtype=np.float32)}
    w = {"q": pack_w(Wq_kernel), "k": pack_w(Wk_kernel),
         "v": pack_w(Wv_kernel)}
    bias = {"q": np.ascontiguousarray(np.asarray(Wq_bias, dtype=np.float32)),
            "k": np.ascontiguousarray(np.asarray(Wk_bias, dtype=np.float32)),
            "v": np.ascontiguousarray(np.asarray(Wv_bias, dtype=np.float32))}

    in_maps = []
    for c in range(N_CORES):
        m = {}
        for t in ("q", "k", "v"):
            if t == "k":
                m["kT"] = (inp["k"][c * B_LOC:(c + 1) * B_LOC]
                           .reshape(B_LOC, KC // 2, 256, DC, P)
                           .transpose(0, 1, 4, 3, 2).astype(BF16_NP)
                           .reshape(B_LOC, KC // 2, P, DC * 256))
            elif t == "q":
                # s-major 512-column blocks: [b][sb][p][c*512+j]
                m["qT"] = (inp["q"][c * B_LOC:(c + 1) * B_LOC]
                           .reshape(B_LOC, 4, 512, DC, P)
                           .transpose(0, 1, 4, 3, 2).astype(BF16_NP)
                           .reshape(B_LOC, 4, P, DC * 512))
            else:
                m[f"{t}T"] = inp[t][c * B_LOC:(c + 1) * B_LOC] \
                    .transpose(0, 2, 1).astype(BF16_NP)
            m[f"w{t}"] = w[t]
        m["b3"] = np.ascontiguousarray(
            np.stack([bias["q"], bias["k"], bias["v"]], axis=1))
        m["bvr"] = bias["v"]
        in_maps.append(m)

    res = run_bass_kernel_spmd(nc, in_maps, list(range(N_CORES)))

    out = np.empty((N_CORES * B_LOC, S, P), dtype=np.float32)
    for c in range(N_CORES):
        out[c * B_LOC:(c + 1) * B_LOC] = (
            res.results[c]["out"].astype(np.float32).transpose(0, 2, 1))
    return out


# revision 44
# speedup vs baseline: 1.1097x; 1.1097x over previous
"""Trainium2 Bass kernel for the AttentionLayer problem.

Math (per batch):
    Q = inp_q @ Wq + bq            [S, d]
    K = inp_k @ Wk + bk            [S, d]
    V = inp_v @ Wv + bv            [S, d]
    sc = Q @ K^T / sqrt(d)         [Sq, Sk]
    S_ = softmax(sc, axis=0)       (over the QUERY axis)
    H = S_ @ V                     [Sq, d]

Schedule (per core, 2 batches, fully software-pipelined):
  The exp chain on ACT (~3.3us per 128-key chunk, 106us/core) and the
  matmul stream on PE (~117us/core at the power-throttled 2.0GHz clock)
  are the two near-critical engines; every phase of batch b is emitted
  interleaved with phases of the other batch so both engines stay busy:

    PE:  [Qproj b0 | kslabs b0 + sc b0(0..8) + Vnat b0 | kslabs b1 +
          sc b0(9..12) | Qproj b1 + sc b0(13..15) | sc b1 + Vnat b1 +
          H b0 | H b1 ]
    ACT: [exp b0 chunks 0..15 | exp b1 chunks 0..15 | last out copy]
    DMA: q0, k0, v0, k1, q1, v1 (the order activations are consumed)

  PSUM (8 banks total):
    P1 (2 x [128,1024] f32 = 4 banks): Qproj-b0 accum halves, then the
       rotating double-buffered scores tiles for both batches.
    P2 (2 x 4KB slots = 4 banks): kps slabs b0 -> V-natural accum b0 ->
       kps slabs b1 -> Qproj-b1 accum halves -> V-natural accum b1 ->
       H accum tiles (one [128,1024] per q-half, both live at once).

  V is projected directly in natural [key, d] layout (lhsT = x-slice,
  ap=128 matmuls: LDWEIGHTS hides under FWL), drained UNNORMALIZED to
  SBUF early (frees PSUM for the next phase), then normalized per key
  chunk k by 1/Z[k] on DVE once chunk k's exp-sum is known.
  H^T[d,q] accumulates k-outer with both q-half tiles live so only the
  last key chunk's matmuls trail the final exp.
Compute dtype bf16 (f32 PSUM accumulate), stats in f32.
"""

import math
import sys

sys.path.insert(0, "/opt/trn_rl_repo")

import ml_dtypes
import numpy as np

BF16_NP = ml_dtypes.bfloat16

import concourse.bass as bass  # noqa: E402
import concourse.tile as tile  # noqa: E402
from concourse import bacc, mybir  # noqa: E402

P = 128          # partitions / head dim d
S = 2048         # sequence length
D = 1024         # model dim
DC = D // P      # D chunks (8)
KC = S // P      # key chunks (16)
B_LOC = 2        # batches per core
N_CORES = 8
SCALE = 1.0 / math.sqrt(P)

F32 = mybir.dt.float32
BF16 = mybir.dt.bfloat16
EXP = mybir.ActivationFunctionType.Exp
COPY = mybir.ActivationFunctionType.Copy

_BUILT = None  # cached (nc,) so repeated kernel() calls reuse the NEFF


def build():
    nc = bacc.Bacc("TRN2", target_bir_lowering=False, debug=False,
                   num_devices=N_CORES)

    dr_in = {}
    dr_in["v"] = nc.dram_tensor("vT", [B_LOC, D, S], BF16,
                                kind="ExternalInput")
    # q host-packed per 512-column s-block: [b][sb][p][c*512+j] =
    # q[b][sb*512+j][c*128+p].  s-major blocks let the Q projection
    # finish its first 1024 output columns after only 1MB of q DMA,
    # so the exp spine starts ~10us earlier than with c-major slabs.
    dr_in["q"] = nc.dram_tensor("qT", [B_LOC, 4, P, DC * 512],
                                BF16, kind="ExternalInput")
    # k host-packed per 256-column slab: [b][sl][p][c*256+j] =
    # k[b][sl*256+j][c*128+p] so each partition reads one contiguous run
    dr_in["k"] = nc.dram_tensor("kT", [B_LOC, KC // 2, P, DC * 256],
                                BF16, kind="ExternalInput")
    dr_w = {t: nc.dram_tensor(f"w{t}", [P, DC * P], BF16,
                              kind="ExternalInput")
            for t in ("q", "k", "v")}
    # biases packed [P, 3] host-side: one DMA instead of three
    # 128-descriptor transfers on the sync queue's critical prefix
    dr_b3 = nc.dram_tensor("b3", [P, 3], F32, kind="ExternalInput")
    dr_bv = nc.dram_tensor("bvr", [P], F32, kind="ExternalInput")
    dr_out = nc.dram_tensor("out", [B_LOC, P, S], BF16, kind="ExternalOutput")

    with tile.TileContext(nc) as tc:
        with (
            tc.tile_pool(name="const", bufs=1) as const,
            tc.tile_pool(name="stream", bufs=4) as stream,
            tc.tile_pool(name="kctp", bufs=16) as kctp,
            tc.tile_pool(name="qtp", bufs=2) as qtp,
            tc.tile_pool(name="ptp", bufs=32) as ptp,
            tc.tile_pool(name="vrawp", bufs=2) as vrawp,
            tc.tile_pool(name="vsp", bufs=32) as vsp,
            tc.tile_pool(name="zzp", bufs=8) as zzp,
            tc.tile_pool(name="recp", bufs=32) as recp,
            tc.tile_pool(name="osb", bufs=2) as osb,
            tc.tile_pool(name="p1", bufs=2, space="PSUM") as p1,
            tc.tile_pool(name="p2", bufs=2, space="PSUM") as p2,
        ):
            w_sb = {}
            for t in ("q", "k", "v"):
                w_sb[t] = const.tile([P, DC, P], BF16, tag=f"w{t}",
                                     name=f"w{t}")
            b3_sb = const.tile([P, 3], F32, tag="b3", name="b3")
            b_sb = {t: b3_sb[:, i:i + 1]
                    for i, t in enumerate(("q", "k", "v"))}

            def load_biases():
                nc.sync.dma_start(b3_sb[:], dr_b3.ap())

            _w_loaded = set()

            def ensure_w(t, eng=None):
                if t in _w_loaded:
                    return
                _w_loaded.add(t)
                (eng or nc.gpsimd).dma_start(
                    w_sb[t][:],
                    dr_w[t].ap().rearrange("p (c e) -> p c e", e=P))

            # V bias as a rank-1 matmul (ones[1,128].T @ bias_row[1,128])
            _vbias_box = []

            def ensure_vbias():
                if not _vbias_box:
                    ones_row = const.tile([1, P], BF16, tag="ones",
                                          name="ones_row")
                    nc.vector.memset(ones_row[:], 1.0)
                    bv_row = const.tile([1, P], BF16, tag="bvr",
                                        name="bv_row")
                    nc.gpsimd.dma_start(
                        bv_row[:],
                        dr_bv.ap().rearrange("(o e) -> o e", o=1))
                    _vbias_box.append((ones_row, bv_row))
                return _vbias_box[0]

            def load_dbl(t, b, cc):
                """One [128, 2, 2048] double D-chunk of v (1MB)."""
                x = stream.tile([P, 2, S], BF16, tag="stream", name="x")
                nc.gpsimd.dma_start(
                    x[:],
                    dr_in[t].ap()[b, cc * 2 * P:(cc + 1) * 2 * P, :]
                    .rearrange("(two p) s -> p two s", two=2))
                return x

            def load_qsb(b, sb, eng=None):
                """One [128, 8, 512] s-block of q (1MB)."""
                x = stream.tile([P, DC, 512], BF16, tag="stream",
                                name="xq")
                (eng or nc.gpsimd).dma_start(
                    x[:],
                    dr_in["q"].ap()[b, sb]
                    .rearrange("p (c s) -> p c s", s=512))
                return x

            # ---------------- Q projection (s-block streamed) ----------
            def emit_qproj_half(b, h, pool, tag, qt, drain_eng,
                                engs=(None, None)):
                """One [128,1024] output half = two 1MB s-blocks, each
                8 accumulating ap512 MMs, then a fused bias-add drain."""
                ensure_w("q")
                acc = pool.tile([P, 1024], F32, tag=tag, name="qacc")
                xs = [load_qsb(b, h * 2 + p_, engs[p_])
                      for p_ in range(2)]
                for p_ in range(2):
                    for c in range(DC):
                        nc.tensor.matmul(
                            acc[:, p_ * 512:(p_ + 1) * 512],
                            lhsT=w_sb["q"][:, c, :],
                            rhs=xs[p_][:, c, :],
                            start=(c == 0), stop=(c == DC - 1))
                sl = qt[:, h * 1024:(h + 1) * 1024]
                if drain_eng == "act":
                    nc.scalar.activation(
                        sl, acc[:],
                        func=mybir.ActivationFunctionType.Identity,
                        bias=b_sb["q"][:])
                else:
                    nc.vector.tensor_scalar_add(sl, acc[:], b_sb["q"][:])

            # ---------------- K slab: [d, 256] = 2 key chunks ----------
            def emit_kslab(b, sl):
                ensure_w("k")
                xk = stream.tile([P, DC, 256], BF16, tag="stream",
                                 name="xk")
                nc.gpsimd.dma_start(
                    xk[:],
                    dr_in["k"].ap()[b, sl]
                    .rearrange("p (c s) -> p c s", s=256))
                kps = p2.tile([P, 256], F32, tag="p2", name="kps")
                for c in range(DC):
                    nc.tensor.matmul(
                        kps[:], lhsT=w_sb["k"][:, c, :], rhs=xk[:, c, :],
                        start=(c == 0), stop=(c == DC - 1))
                kct = kctp.tile([P, 256], BF16, tag="kct", name="kct")
                nc.vector.tensor_scalar_add(kct[:], kps[:], b_sb["k"][:])
                return kct

            # ---------------- scores chunk + exp ----------------------
            def emit_sc(b, st, j, halves=(0, 1)):
                """One key chunk j of scores^T + exp + Z accumulate."""
                kct = st.kcts[j // 2]
                lhsT = kct[:, (j % 2) * P:(j % 2 + 1) * P]
                if 0 in halves:
                    pt = ptp.tile([P, S], BF16, tag="pt", name="pt")
                    zz = zzp.tile([P, 2], F32, tag="zz", name="zz")
                    st.pts.append(pt)
                    st.zzs.append(zz)
                else:
                    pt, zz = st.pts[j], st.zzs[j]
                for h in halves:
                    sc = p1.tile([P, 1024], F32, tag="p1", name="sc")
                    for s2 in range(2):
                        nc.tensor.matmul(
                            sc[:, s2 * 512:(s2 + 1) * 512],
                            lhsT=lhsT,
                            rhs=st.qt[:, h * 1024 + s2 * 512:
                                      h * 1024 + (s2 + 1) * 512],
                            start=True, stop=True)
                    nc.scalar.activation(
                        pt[:, h * 1024:(h + 1) * 1024], sc[:],
                        func=EXP, scale=SCALE, accum_out=zz[:, h:h + 1])

            def emit_rec(st):
                """Emit 1/Z for the next pending chunk (DVE)."""
                zz = st.zzs[len(st.recs)]
                rec = recp.tile([P, 1], F32, tag="rec", name="rec")
                nc.vector.tensor_reduce(
                    rec[:], zz[:], axis=mybir.AxisListType.X,
                    op=mybir.AluOpType.add)
                nc.vector.reciprocal(rec[:], rec[:])
                st.recs.append(rec)

            # ---------------- V natural projection ---------------------
            def emit_v_dbl_mms(b, st, cc, x):
                """V-natural MMs for one double D-chunk: 32 ap128 MMs."""
                ensure_w("v")
                for two in range(2):
                    c = cc * 2 + two
                    for g in range(KC):
                        nc.tensor.matmul(
                            st.v_ps[g // 8][:, g % 8, :],
                            lhsT=x[:, two, g * P:(g + 1) * P],
                            rhs=w_sb["v"][:, c, :],
                            start=(c == 0 and (g % 8) % 4 == 0),
                            stop=False)

            def emit_v_bias(st):
                ones_row, bv_row = ensure_vbias()
                for g in range(KC):
                    nc.tensor.matmul(
                        st.v_ps[g // 8][:, g % 8, :],
                        lhsT=ones_row[:], rhs=bv_row[:],
                        start=False, stop=True)

            def emit_v_drain(st):
                """Unnormalized PSUM->SBUF drain (frees P2 early)."""
                vraw = vrawp.tile([P, KC, P], BF16, tag="vraw",
                                  name="vraw")
                for half in range(2):
                    nc.vector.tensor_copy(
                        vraw[:, half * 8:(half + 1) * 8, :],
                        st.v_ps[half][:])
                st.vraw = vraw

            def emit_norm(st, k):
                """vs[k] = vraw[k] * (1/Z[k]) on DVE (4x mode)."""
                while len(st.recs) <= k:
                    emit_rec(st)
                vs = vsp.tile([P, P], BF16, tag="vs", name="vs")
                nc.vector.tensor_scalar_mul(vs[:], st.vraw[:, k, :],
                                            st.recs[k][:])
                st.vss.append(vs)

            # ---------------- H accumulation ---------------------------
            def emit_h_alloc(st):
                st.hts = [p2.tile([P, 1024], F32, tag="p2", name="ht")
                          for _ in range(2)]

            def emit_h_k(st, k):
                for qh in range(2):
                    for s2 in range(2):
                        nc.tensor.matmul(
                            st.hts[qh][:, s2 * 512:(s2 + 1) * 512],
                            lhsT=st.vss[k][:],
                            rhs=st.pts[k][:, qh * 1024 + s2 * 512:
                                          qh * 1024 + (s2 + 1) * 512],
                            start=(k == 0), stop=(k == KC - 1))

            def emit_ht_drain(b, st, qh, engine):
                out_sb = osb.tile([P, 1024], BF16, tag="osb",
                                  name="out_sb")
                if engine == "act":
                    nc.scalar.activation(out_sb[:], st.hts[qh][:],
                                         func=COPY)
                else:
                    nc.vector.tensor_copy(out_sb[:], st.hts[qh][:])
                nc.sync.dma_start(
                    dr_out.ap()[b][:, qh * 1024:(qh + 1) * 1024],
                    out_sb[:])

            class St:   # per-batch bookkeeping
                def __init__(self):
                    self.qt = None
                    self.kcts = []
                    self.pts = []
                    self.zzs = []
                    self.recs = []
                    self.v_ps = None
                    self.vraw = None
                    self.vss = []
                    self.hts = None

            st0, st1 = St(), St()

            def sc_emit(st, b):
                """Emit the next pending scores chunk of batch b (4 MMs
                on PE + 2 exps on ACT), plus the lag-2 1/Z on DVE."""
                j = len(st.pts)
                emit_sc(b, st, j)
                if j >= 2:
                    emit_rec(st)

            # ================= EMISSION SEQUENCE =======================
            # Phase A: b0 Q projection from s-major blocks.  The LAST
            # q block rides the (slower but parallel) sync queue so q0
            # completes ~3us before the serial gpsimd stream could
            # deliver it; both qt halves drain on separate engines (ACT
            # idle pre-spine) so the exp spine starts right after.
            ensure_w("q", nc.sync)
            load_biases()   # MUST precede its readers (the qt drains);
            ensure_w("k", nc.sync)  # w_k not consumed until ~24us
            st0.qt = qtp.tile([P, S], BF16, tag="qt", name="qt0")
            emit_qproj_half(0, 0, p1, "p1", st0.qt, "act")
            emit_qproj_half(0, 1, p1, "p1", st0.qt, "dve")
            st0.kcts.append(emit_kslab(0, 0))
            sc_emit(st0, 0)                # chunk 0

            # Phase B: b0 K slabs (DMA-paced) + sc chunks 1..7 (ACT-
            # paced) + V0 natural MMs streaming behind the v0 DMAs.
            # sc chunk j+1's PSUM slot frees when exp j (same half)
            # retires, so sc emissions are spaced to match; all kps
            # tiles are allocated BEFORE the v_ps tiles so the shared
            # P2 slot rotation matches temporal use.
            xv00 = None
            for j in range(1, 8):
                st0.kcts.append(emit_kslab(0, j))
                if j <= 4:
                    sc_emit(st0, 0)        # chunks 1..4
                elif j == 5:
                    # prefetch v0's first double ahead of k slabs 6,7
                    # (their kct is not consumed until chunks 12..15)
                    xv00 = load_dbl("v", 0, 0)
            st0.v_ps = [p2.tile([P, 8, P], F32, tag="p2", name="v_ps")
                        for _ in range(2)]
            for cc in range(4):            # v0 doubles arrive 30..43us
                x = xv00 if cc == 0 else load_dbl("v", 0, cc)
                emit_v_dbl_mms(0, st0, cc, x)
                if cc < 3:
                    sc_emit(st0, 0)        # chunks 5..7
            emit_v_bias(st0)
            emit_v_drain(st0)

            # Phase C: b1 Q projection [47..58us] + sc b0 chunks 8..11,
            # finely interleaved between the four 1MB q1 s-blocks.
            # q1 is loaded BEFORE k1 so PE has dense work here (kct1 is
            # not needed until ~64us); qacc1 tiles WAR the v_ps0 drains.
            st1.qt = qtp.tile([P, S], BF16, tag="qt", name="qt1")
            sc_emit(st0, 0)                # chunk 8
            qacc1 = []
            for h in range(2):
                acc = p2.tile([P, 1024], F32, tag="p2", name="qacc1")
                qacc1.append(acc)
                for p_ in range(2):
                    x = load_qsb(1, h * 2 + p_)
                    for c in range(DC):
                        nc.tensor.matmul(
                            acc[:, p_ * 512:(p_ + 1) * 512],
                            lhsT=w_sb["q"][:, c, :], rhs=x[:, c, :],
                            start=(c == 0), stop=(c == DC - 1))
                    if len(st0.pts) < 12:
                        sc_emit(st0, 0)    # chunks 9,10,11
                nc.vector.tensor_scalar_add(
                    st1.qt[:, h * 1024:(h + 1) * 1024], acc[:],
                    b_sb["q"][:])
            # normalize b0 V rows 0..9 (recs ready well before this
            # point in the DVE stream)
            for k in range(10):
                emit_norm(st0, k)

            # Phase D: b1 K slabs [50..60us] + sc b0 chunks 12,13
            xv10 = None
            for j in range(8):
                st1.kcts.append(emit_kslab(1, j))
                if j in (1, 4):
                    sc_emit(st0, 0)        # chunks 12,13
                elif j == 5:
                    xv10 = load_dbl("v", 1, 0)   # prefetch (as v0's)

            # Phase E: V1 streaming + sc b0 tail + sc b1 head.
            # Order keeps the exp spine seamless across the batch
            # boundary: b1 chunk 0 must be computed right after b0
            # chunk 15's PSUM slot frees.
            x = xv10
            st1.v_ps = [p2.tile([P, 8, P], F32, tag="p2", name="v_ps1")
                        for _ in range(2)]
            emit_v_dbl_mms(1, st1, 0, x)
            sc_emit(st0, 0)                # chunk 14
            x = load_dbl("v", 1, 1)
            emit_v_dbl_mms(1, st1, 1, x)
            sc_emit(st0, 0)                # chunk 15
            sc_emit(st1, 1)                # b1 chunk 0
            x = load_dbl("v", 1, 2)
            emit_v_dbl_mms(1, st1, 2, x)
            sc_emit(st1, 1)                # b1 chunk 1
            x = load_dbl("v", 1, 3)
            emit_v_dbl_mms(1, st1, 3, x)
            emit_v_bias(st1)
            sc_emit(st1, 1)                # b1 chunk 2
            emit_v_drain(st1)
            for k in range(10, KC):        # finish b0 normalizes
                emit_norm(st0, k)

            # Phase F: H0 accumulation k-outer, woven with b1 sc chunks
            # 3..9 (sc first in each pair: ACT is the spine, PE may
            # briefly wait on the scores slot WAR but never starves ACT)
            emit_h_alloc(st0)
            for k in range(KC):
                if k % 2 == 0 and k < 14:
                    sc_emit(st1, 1)        # b1 chunks 3..9
                emit_h_k(st0, k)
            # b1 norms 0,1 BEFORE the b0 output drains in the DVE
            # stream so H1's start is not delayed behind the copies
            emit_norm(st1, 0)
            emit_norm(st1, 1)
            emit_ht_drain(0, st0, 0, "dve")
            emit_ht_drain(0, st0, 1, "dve")

            # Phase G: H1 woven with b1 sc chunks 10..15 (tail)
            emit_h_alloc(st1)
            for k in range(KC):
                if k % 2 == 0 and k < 12:
                    sc_emit(st1, 1)        # b1 chunks 10..15
                if k >= 2:
                    emit_norm(st1, k)
                emit_h_k(st1, k)
            emit_ht_drain(1, st1, 0, "dve")
            emit_ht_drain(1, st1, 1, "act")

    nc.compile()
    return nc


def _get_nc():
    global _BUILT
    if _BUILT is None:
        _BUILT = build()
    return _BUILT


def pack_w(wk):
    """[D, P] f32 -> [P, DC*P] bf16 in the on-chip [p, c, e] layout."""
    wk = np.asarray(wk, dtype=np.float32)
    return np.ascontiguousarray(
        wk.reshape(DC, P, P).transpose(1, 0, 2).reshape(P, DC * P)
    ).astype(BF16_NP)


def kernel(inp_q, inp_k, inp_v, Wq_kernel, Wq_bias, Wk_kernel, Wk_bias,
           Wv_kernel, Wv_bias):
    from concourse.bass_utils import run_bass_kernel_spmd

    nc = _get_nc()

    inp = {"q": np.asarray(inp_q, dtype=np.float32),
           "k": np.asarray(inp_k, dtype=np.float32),
           "v": np.asarray(inp_v, dtype=np.float32)}
    w = {"q": pack_w(Wq_kernel), "k": pack_w(Wk_kernel),
         "v": pack_w(Wv_kernel)}
    bias = {"q": np.ascontiguousarray(np.asarray(Wq_bias, dtype=np.float32)),
            "k": np.ascontiguousarray(np.asarray(Wk_bias, dtype=np.float32)),
            "v": np.ascontiguousarray(np.asarray(Wv_bias, dtype=np.float32))}

    in_maps = []
    for c in range(N_CORES):
        m = {}
        for t in ("q", "k", "v"):
            if t == "k":
                m["kT"] = (inp["k"][c * B_LOC:(c + 1) * B_LOC]
                           .reshape(B_LOC, KC // 2, 256, DC, P)
                           .transpose(0, 1, 4, 3, 2).astype(BF16_NP)
                           .reshape(B_LOC, KC // 2, P, DC * 256))
            elif t == "q":
                # s-major 512-column blocks: [b][sb][p][c*512+j]
                m["qT"] = (inp["q"][c * B_LOC:(c + 1) * B_LOC]
                           .reshape(B_LOC, 4, 512, DC, P)
                           .transpose(0, 1, 4, 3, 2).astype(BF16_NP)
                           .reshape(B_LOC, 4, P, DC * 512))
            else:
                m[f"{t}T"] = inp[t][c * B_LOC:(c + 1) * B_LOC] \
                    .transpose(0, 2, 1).astype(BF16_NP)
            m[f"w{t}"] = w[t]
        m["b3"] = np.ascontiguousarray(
            np.stack([bias["q"], bias["k"], bias["v"]], axis=1))
        m["bvr"] = bias["v"]
        in_maps.append(m)

    res = run_bass_kernel_spmd(nc, in_maps, list(range(N_CORES)))

    out = np.empty((N_CORES * B_LOC, S, P), dtype=np.float32)
    for c in range(N_CORES):
        out[c * B_LOC:(c + 1) * B_LOC] = (
            res.results[c]["out"].astype(np.float32).transpose(0, 2, 1))
    return out
